# revision 1
# baseline (speedup 1.0000x reference)
"""Trainium2 Bass kernel for the 2-layer CIN (Compressed Interaction Network).

Math (per batch b, reference):
  x1[b,h,k] = sum_{i,j} W1[h,i,j] * x[b,i,k] * x[b,j,k] + b1[h]
  x2[b,h,k] = sum_{i,j} W2[h,i,j] * x1[b,i,k] * x[b,j,k] + b2[h]
  out[b, :] = [sum_k x1[b,:,k], sum_k x2[b,:,k]]          # [B, 256]

Device strategy (pure data parallel over 8 cores, 256 batches each):
  - Columns col=(b_lo 4, k 32) live on the 128 SBUF partitions; 64 col-tiles.
  - Z[col, pq] holds symmetry-folded outer products a_p * a_{(p+d)%26} with
    pq=(d parity-split 2x8, p padded to 32) = 512 rows; the last row is 1.0 to
    carry b1 through the x1 matmul. Built with 2 sliding-window DVE multiplies
    per tile in bf16 (each op is the sole producer of two 128-col chunks).
  - Z transposed 128x128 via DMA-xbar (SBUF->SBUF, bf16) into ZT[pq, col].
  - x1[h, col] = C^T @ ZT with host-folded symmetric W1 (4 accumulating
    matmuls per 512 columns).
  - x1T via PE transpose; then per col-tile two selector matmuls sharing one
    weight load: G2[i,(bl,j)] (host-built block-diag A selector) and
    out1[i,b'] (0/1 window selector, PSUM-accumulated over 32 tiles).
  - out2[h,b] = 26 accumulating matmuls over j with host-permuted W2; b2 is
    added during the PSUM->SBUF copy.
"""

import dataclasses
import os
import sys

sys.path.insert(0, "/opt/trn_rl_repo")

import numpy as np
import ml_dtypes

import concourse.bass as bass
import concourse.tile as tile
from concourse import bacc
from concourse import mybir
from concourse.bass_utils import run_bass_kernel_spmd

BF = ml_dtypes.bfloat16

B, M, K, H = 2048, 26, 32, 128
NC = 8
BS = B // NC        # 256 batches per core
NT = BS // 4        # 64 col tiles
PQ = 512            # padded pair dim (4 chunks of 128)
AE = 48             # per-tile stride in a_ext / a_ext2

F32 = mybir.dt.float32
BF16 = mybir.dt.bfloat16


def _sl(ap, ap_dims, extra_off=0):
    """Raw AP with custom free dims [(step, count), ...]."""
    return dataclasses.replace(
        ap, offset=ap.offset + extra_off,
        ap=[list(ap.ap[0])] + [[s, c] for s, c in ap_dims])


def build_nc(debug_dump=False):
    nc = bacc.Bacc("TRN2", target_bir_lowering=False, debug=False,
                   num_devices=NC)

    dr = lambda n, shp, dt: nc.dram_tensor(n, shp, dt, kind="ExternalInput").ap()
    apad_d = dr("apad", [128, NT * 32], BF16)
    aext_d = dr("aext", [128, NT * AE], BF16)
    aex2_d = dr("aex2", [128, NT * AE], BF16)
    as_d = dr("asd", [128, NT * 108], BF16)
    c_d = dr("c_w", [128, PQ], BF16)
    w2_d = dr("w2p", [128, 26 * 128], BF16)
    idb_d = dr("idb", [128, 128], BF16)
    idf_d = dr("idf", [128, 128], F32)
    b2_d = dr("b2s", [128, 1], F32)
    res_d = nc.dram_tensor("res", [BS, 256], F32, kind="ExternalOutput").ap()
    dbg = None
    if debug_dump:
        dbg = {
            "ztb": nc.dram_tensor("d_ztb", [128, 4 * NT * 128], BF16,
                                  kind="ExternalOutput").ap(),
            "x1t": nc.dram_tensor("d_x1t", [128, NT * 128], BF16,
                                  kind="ExternalOutput").ap(),
            "g2sb": nc.dram_tensor("d_g2sb", [128, NT * 108], BF16,
                                   kind="ExternalOutput").ap(),
            "zbuf": nc.dram_tensor("d_zbuf", [128, NT * PQ], BF16,
                                   kind="ExternalOutput").ap(),
        }

    with tile.TileContext(nc, trace_sim=False) as tc:
        _body(nc, apad_d, aext_d, aex2_d, as_d, c_d, w2_d, idb_d, idf_d,
              b2_d, res_d, dbg)
    nc.compile()
    return nc


def _body(nc, apad_d, aext_d, aex2_d, as_d, c_d, w2_d, idb_d, idf_d,
          b2_d, res_d, dbg=None):
    sb = lambda n, f, dt: nc.alloc_sbuf_tensor(n, [128, f], dt).ap()
    ps = lambda n, f, dt: nc.alloc_psum_tensor(n, [128, f], dt).ap()

    apad = sb("apad_s", NT * 32, BF16)
    aext = sb("aext_s", NT * AE, BF16)
    aex2 = sb("aex2_s", NT * AE, BF16)
    asb = sb("asb", NT * 108, BF16)
    zbuf = sb("zbuf", NT * PQ, BF16)
    ztb = sb("ztb", 4 * NT * 128, BF16)
    x1r = sb("x1r", 1024, BF16)
    x1t = sb("x1t", NT * 128, BF16)
    g2sb = sb("g2sb", NT * 108, BF16)
    csb = sb("csb", PQ, BF16)
    w2p = sb("w2p_s", 26 * 128, BF16)
    idb = sb("idb_s", 128, BF16)
    idf = sb("idf_s", 128, F32)
    b2s = sb("b2s_s", 1, F32)
    out1s = sb("out1s", 256, F32)
    out2s = sb("out2s", 256, F32)
    ress = sb("ress", 512, F32)

    x1p = [ps(f"x1p{i}", 512, F32) for i in range(2)]
    xtp = [ps(f"xtp{i}", 128, BF16) for i in range(2)]
    g2p = [ps(f"g2p{i}", 432, F32) for i in range(2)]
    accp = ps("accp", 256, F32)
    ftp = ps("ftp", 128, F32)

    # ---- loads (A-tensors in 4 chunks each for pipelining) ----
    for g in range(4):
        s = slice(g * 16 * 32, (g + 1) * 16 * 32)
        nc.scalar.dma_start(apad[:, s], apad_d[:, s])
        s = slice(g * 16 * AE, (g + 1) * 16 * AE)
        nc.scalar.dma_start(aext[:, s], aext_d[:, s])
        nc.scalar.dma_start(aex2[:, s], aex2_d[:, s])
        s = slice(g * 16 * 108, (g + 1) * 16 * 108)
        nc.scalar.dma_start(asb[:, s], as_d[:, s])
    nc.scalar.dma_start(csb, c_d)
    nc.scalar.dma_start(w2p, w2_d)
    nc.scalar.dma_start(idb, idb_d)
    nc.scalar.dma_start(idf, idf_d)
    nc.scalar.dma_start(b2s, b2_d)

    # ---- Z build: 2 sliding-window DVE multiplies per tile ----
    for t in range(NT):
        op1 = apad[:, t * 32: t * 32 + 32][:, None, :].broadcast_to(
            (128, 8, 32))
        op2e = _sl(aext, [(2, 8), (1, 32)], extra_off=t * AE)
        op2o = _sl(aex2, [(2, 8), (1, 32)], extra_off=t * AE)
        oute = zbuf[:, t * PQ: t * PQ + 256].rearrange(
            "p (a b) -> p a b", b=32)
        outo = zbuf[:, t * PQ + 256: t * PQ + 512].rearrange(
            "p (a b) -> p a b", b=32)
        nc.vector.tensor_mul(oute, op1, op2e)
        nc.vector.tensor_mul(outo, op1, op2o)

    # ---- Z transpose via DMA xbar (each chunk has a single producer) ----
    for t in range(NT):
        for c in range(4):
            nc.sync.dma_start(
                ztb[:, c * NT * 128 + t * 128: c * NT * 128 + (t + 1) * 128],
                zbuf[:, t * PQ + c * 128: t * PQ + (c + 1) * 128],
                transpose=True)

    # ---- per round r (512 cols = 4 tiles): x1 matmuls + copy + per-tile
    #      transpose, G2/out1 selector matmuls ----
    for r in range(16):
        p = x1p[r % 2]
        for c in range(4):
            nc.tensor.matmul(
                p, csb[:, c * 128:(c + 1) * 128],
                ztb[:, c * NT * 128 + r * 512: c * NT * 128 + (r + 1) * 512],
                start=(c == 0), stop=(c == 3), skip_group_check=True)
        xs = x1r[:, (r % 2) * 512:(r % 2 + 1) * 512]
        nc.scalar.copy(xs, p)
        for t in range(4 * r, 4 * r + 4):
            tau, g, half, t32 = t % 4, t // 4, t // 32, t % 32
            nc.tensor.transpose(xtp[t % 2], xs[:, tau * 128:(tau + 1) * 128],
                                idb)
            lhs = x1t[:, t * 128:(t + 1) * 128]
            nc.scalar.copy(lhs, xtp[t % 2])
            nc.tensor.matmul(g2p[g % 2][:, tau * 108:(tau + 1) * 108],
                             lhs, asb[:, t * 108:(t + 1) * 108],
                             start=True, stop=True, skip_group_check=True)
            if tau == 3:
                nc.vector.tensor_copy(g2sb[:, g * 432:(g + 1) * 432],
                                      g2p[g % 2])

    # ---- out2: 26 accumulating matmuls over j ----
    for j in range(26):
        rhs = _sl(g2sb, [(108, NT), (26, 4)], extra_off=j)
        nc.tensor.matmul(accp, w2p[:, j * 128:(j + 1) * 128],
                         rhs, start=(j == 0), stop=(j == 25),
                         skip_group_check=True)

    # ---- finals: out1 from g2sb cols, b2 add, transpose to [b, h] ----
    o1src = _sl(g2sb, [(108, NT), (1, 4)], extra_off=104)
    nc.vector.tensor_copy(out1s.rearrange("p (t c) -> p t c", c=4), o1src)
    nc.vector.tensor_scalar(out2s, accp, b2s, None,
                            mybir.AluOpType.add)
    for u in range(2):
        nc.tensor.transpose(ftp, out1s[:, u * 128:(u + 1) * 128], idf)
        nc.vector.tensor_copy(ress[:, u * 256: u * 256 + 128], ftp)
        nc.tensor.transpose(ftp, out2s[:, u * 128:(u + 1) * 128], idf)
        nc.vector.tensor_copy(ress[:, u * 256 + 128: u * 256 + 256], ftp)
        nc.scalar.dma_start(res_d[u * 128:(u + 1) * 128, :],
                          ress[:, u * 256:(u + 1) * 256])
    if dbg is not None:
        nc.gpsimd.dma_start(dbg["ztb"], ztb)
        nc.gpsimd.dma_start(dbg["x1t"], x1t)
        nc.gpsimd.dma_start(dbg["g2sb"], g2sb)
        nc.gpsimd.dma_start(dbg["zbuf"], zbuf)


def host_prep_weights(W1, b1, W2, b2):
    # C matrix [512, 128]: rows (parity-block, m, p32); last row carries b1.
    C = np.zeros((PQ, H), dtype=np.float32)
    for d in range(14):
        base = (d // 2) * 32 if d % 2 == 0 else 256 + ((d - 1) // 2) * 32
        for p in range(26):
            q = (p + d) % 26
            if d == 0:
                coeff = W1[:, p, p]
            elif d == 13:
                coeff = 0.5 * (W1[:, p, q] + W1[:, q, p])
            else:
                coeff = W1[:, p, q] + W1[:, q, p]
            C[base + p, :] = coeff
    C[511, :] = b1
    csb = C.reshape(4, 128, H).transpose(1, 0, 2).reshape(128, PQ)
    w2p = W2.transpose(1, 2, 0).reshape(128, 26 * 128)
    return (csb.astype(BF), w2p.astype(BF),
            np.eye(128, dtype=np.float32).astype(BF),
            np.eye(128, dtype=np.float32),
            (32.0 * b2[:, None]).astype(np.float32))


def host_prep_inputs(inputs):
    """Per-core A layouts (pure relayout/padding of the input tensor)."""
    a = inputs.reshape(NC, NT, 4, 26, 32).transpose(0, 2, 4, 1, 3)
    ab = np.ascontiguousarray(a).astype(BF)      # [NC, 4, 32, NT, 26] -> view
    ab = ab.reshape(NC, 128, NT, 26)
    apad = np.zeros((NC, 128, NT, 32), dtype=BF)
    apad[:, :, :, 0:26] = ab
    apad[:, :, :, 31] = 1.0
    aext = np.zeros((NC, 128, NT, AE), dtype=BF)
    aext[:, :, :, 0:26] = ab
    aext[:, :, :, 26:39] = ab[:, :, :, 0:13]
    aex2 = np.zeros((NC, 128, NT, AE), dtype=BF)
    aex2[:, :, :, 0:47] = aext[:, :, :, 1:48]
    aex2[:, :, :, 45] = 1.0
    asd = np.zeros((NC, 128, NT, 108), dtype=BF)
    for bl in range(4):
        asd[:, bl * 32:(bl + 1) * 32, :, bl * 26:(bl + 1) * 26] = \
            ab[:, bl * 32:(bl + 1) * 32]
        asd[:, bl * 32:(bl + 1) * 32, :, 104 + bl] = 1.0
    rs = lambda x: np.ascontiguousarray(x.reshape(NC, 128, -1))
    return rs(apad), rs(aext), rs(aex2), rs(asd)


_nc_cache = {}


def kernel(inputs, W1, b1, W2, b2):
    inputs = np.ascontiguousarray(np.asarray(inputs, dtype=np.float32))
    W1 = np.asarray(W1, dtype=np.float32)
    b1 = np.asarray(b1, dtype=np.float32)
    W2 = np.asarray(W2, dtype=np.float32)
    b2 = np.asarray(b2, dtype=np.float32)

    csb, w2p, idb, idf, b2s = host_prep_weights(W1, b1, W2, b2)
    apad, aext, aex2, asd = host_prep_inputs(inputs)

    if "nc" not in _nc_cache:
        _nc_cache["nc"] = build_nc()
    nc = _nc_cache["nc"]

    in_maps = []
    for c in range(NC):
        in_maps.append({
            "apad": apad[c], "aext": aext[c], "aex2": aex2[c], "asd": asd[c],
            "c_w": csb, "w2p": w2p,
            "idb": idb, "idf": idf, "b2s": b2s,
        })
    r = run_bass_kernel_spmd(nc, in_maps, core_ids=list(range(NC)),
                             trace=bool(int(os.environ.get("K_TRACE", "0"))))
    out = np.concatenate([r.results[c]["res"] for c in range(NC)], axis=0)
    if r.exec_time_ns is not None:
        kernel.last_exec_ns = r.exec_time_ns
    kernel.last_results = r
    return out


kernel.last_exec_ns = None
kernel.last_results = None


if __name__ == "__main__":
    import reference
    inp = {k: np.asarray(v) for k, v in reference.setup_inputs().items()}
    expected = np.asarray(reference.reference(**inp))
    got = kernel(**inp)
    err = np.abs(got - expected).max()
    rel = err / np.abs(expected).max()
    print("max abs err:", err, "rel:", rel)

